# revision 12
# baseline (speedup 1.0000x reference)
import sys, os
sys.path.insert(0, "/opt/trn_rl_repo")
import numpy as np

import concourse.bass as bass
import concourse.bacc as bacc
import concourse.mybir as mybir
from concourse.tile import TileContext
from concourse.bass_utils import run_bass_kernel_spmd

F32 = mybir.dt.float32
F32R = mybir.dt.float32r

N_CORES = 8
B = 256
B_LOC = B // N_CORES  # 32

# matmul operand dtype mode: "f32r" (full-rate fp32), "f32" (4x slower), "bf16"
MM_MODE = os.environ.get("CAPS_MM_MODE", "f32r")

_cache = {}


def _mm_cast(ap):
    return ap


def build_kernel():
    nc = bacc.Bacc("TRN2", target_bir_lowering=False, debug=False, num_devices=N_CORES)

    x81d = nc.declare_dram_parameter("x81d", [81, 552 * B_LOC], F32, isOutput=False)
    w1T = nc.declare_dram_parameter("w1T", [81, 256], F32, isOutput=False)
    b1 = nc.declare_dram_parameter("b1", [128, 2], F32, isOutput=False)
    w2T = nc.declare_dram_parameter("w2T", [81, 256, 256], F32, isOutput=False)
    b2 = nc.declare_dram_parameter("b2", [128, 2], F32, isOutput=False)
    u_out = nc.declare_dram_parameter("u_out", [256, 36 * B_LOC], F32, isOutput=True)

    with TileContext(nc) as tc:
        with (
            tc.tile_pool(name="const", bufs=1) as cpool,
            tc.tile_pool(name="x81p", bufs=1) as x81pool,
            tc.tile_pool(name="xsb", bufs=1) as xpool,
            tc.tile_pool(name="usb", bufs=1) as upool,
            tc.tile_pool(name="w2p", bufs=4) as w2pool,
        ):
            # ---- constants ----
            w1_sb = cpool.tile([81, 256], F32R)
            nc.gpsimd.dma_start(out=w1_sb[:, :], in_=w1T[:, :])
            b1_sb = cpool.tile([128, 2], F32)
            nc.sync.dma_start(out=b1_sb[:, :], in_=b1[:, :])
            b2_sb = cpool.tile([128, 2], F32)
            nc.sync.dma_start(out=b2_sb[:, :], in_=b2[:, :])

            # ---- conv1 im2col: X81[p=(ky,kx), q'=(oy*28+ox), b] ----
            # partition p reads imgT.flat[(ky*28+kx)*32 : +552*32]
            x81 = x81pool.tile([81, 552 * B_LOC], F32R)
            nc.gpsimd.dma_start(out=x81[:, :], in_=x81d[:, :])

            # ---- conv1: out[co, oy, ox, b], K=81 ----
            x_sb = xpool.tile([128, 2 * 20 * 20 * B_LOC], F32R)  # free=(cot,y,x,b)
            x_r = x_sb.rearrange(
                "p (t y x b) -> p t y x b", t=2, y=20, x=20, b=B_LOC
            )
            x81_r = x81.rearrange("p (q b) -> p q b", b=B_LOC)
            with tc.tile_pool(name="ps1", bufs=4, space="PSUM") as ps1:
                for oy in range(20):
                    for h in range(2):
                        q0 = oy * 28 + h * 10
                        rhs = x81_r[:, q0 : q0 + 10, :]
                        for cot in range(2):
                            pt = ps1.tile([128, 10 * B_LOC], F32)
                            nc.tensor.matmul(
                                pt[:, :],
                                _mm_cast(w1_sb[:, cot * 128 : cot * 128 + 128]),
                                _mm_cast(rhs),
                                start=True,
                                stop=True,
                            )
                            # bias + relu -> SBUF
                            nc.scalar.activation(
                                x_r[:, cot, oy, h * 10 : h * 10 + 10, :],
                                pt[:, :],
                                mybir.ActivationFunctionType.Relu,
                                bias=b1_sb[:, cot : cot + 1],
                            )

            # ---- conv2: 9x9 s2, K=256 (2 tiles), out u[co, s=(6x6), b] ----
            u_sb = upool.tile([128, 2 * 36 * B_LOC], F32)  # free=(cot, s, b)
            u_r = u_sb.rearrange("p (t s b) -> p t s b", t=2, s=36, b=B_LOC)
            # strided view of x for stride-2 conv taps
            x_v = x_sb.rearrange(
                "p (t y2 yp x2 xp b) -> p t yp xp y2 x2 b",
                t=2, y2=10, yp=2, x2=10, xp=2, b=B_LOC,
            )
            with tc.tile_pool(name="ps2", bufs=1, space="PSUM") as ps2:
                pts = {}
                for cot in range(2):
                    for j in range(3):
                        pts[(cot, j)] = ps2.tile([128, 6 * 2 * B_LOC], F32, tag=f"acc{cot}{j}", name=f"acc{cot}{j}")
                for pos in range(81):
                    ky, kx = pos // 9, pos % 9
                    wt = w2pool.tile([128, 2 * 256], F32R, tag="w2")
                    wsrc = w2T.rearrange("pos (ct p) co -> pos p ct co", ct=2, p=128)
                    nc.gpsimd.dma_start(out=wt[:, :], in_=wsrc[pos, :, :, :])
                    wt_r = wt.rearrange("p (ct co) -> p ct co", ct=2)
                    for cint in range(2):
                        for cot in range(2):
                            lhsT = wt_r[:, cint, cot * 128 : cot * 128 + 128]
                            for j in range(3):
                                rhs = x_v[
                                    :, cint, ky % 2, kx % 2,
                                    ky // 2 : ky // 2 + 6,
                                    kx // 2 + 2 * j : kx // 2 + 2 * j + 2,
                                    :,
                                ]
                                nc.tensor.matmul(
                                    pts[(cot, j)][:, :],
                                    _mm_cast(lhsT),
                                    _mm_cast(rhs),
                                    start=(pos == 0 and cint == 0),
                                    stop=(pos == 80 and cint == 1),
                                )
                # epilogue: bias add, to u_sb; s=(oy*6+ox), chunk j covers ox'(2j,2j+1)
                u_v = u_sb.rearrange(
                    "p (t oy j ox b) -> p t oy j ox b", t=2, oy=6, j=3, ox=2, b=B_LOC
                )
                for cot in range(2):
                    for j in range(3):
                        src_p = pts[(cot, j)].rearrange(
                            "p (oy ox b) -> p oy ox b", oy=6, ox=2, b=B_LOC
                        )
                        nc.vector.tensor_scalar_add(
                            u_v[:, cot, :, j, :, :],
                            src_p[:, :, :, :],
                            b2_sb[:, cot : cot + 1],
                        )

            # ---- store u ----
            uo = u_out.rearrange("(t p) sb -> p t sb", t=2, p=128)
            nc.sync.dma_start(out=uo[:, :, :], in_=u_sb[:, :])

    nc.finalize()
    return nc


def _get_nc():
    if "nc" not in _cache:
        _cache["nc"] = build_kernel()
    return _cache["nc"]


def _softmax(x, axis):
    m = np.max(x, axis=axis, keepdims=True)
    e = np.exp(x - m)
    return e / np.sum(e, axis=axis, keepdims=True)


def _squash(x, axis=-1):
    sq = np.sum(x * x, axis=axis, keepdims=True)
    return sq * x / ((1.0 + sq) * np.sqrt(sq))


def kernel(data, conv1_w, conv1_b, pc_w, pc_b, W_dc,
           dec_w1, dec_b1, dec_w2, dec_b2, dec_w3, dec_b3,
           _return_results=False, _trace=False):
    nc = _get_nc()

    w1T_np = np.ascontiguousarray(
        conv1_w.reshape(256, 81).T.astype(np.float32))
    b1_np = np.ascontiguousarray(conv1_b.reshape(2, 128).T.astype(np.float32))
    w2T_np = np.ascontiguousarray(
        pc_w.reshape(256, 256, 9, 9).transpose(2, 3, 1, 0).reshape(81, 256, 256)
        .astype(np.float32))
    b2_np = np.ascontiguousarray(pc_b.reshape(2, 128).T.astype(np.float32))

    in_maps = []
    for k in range(N_CORES):
        sl = data[k * B_LOC : (k + 1) * B_LOC].reshape(B_LOC, 784)
        imgT_np = np.ascontiguousarray(sl.T.astype(np.float32))
        flat = imgT_np.reshape(-1)
        st = flat.strides[0]
        x81_np = np.ascontiguousarray(
            np.lib.stride_tricks.as_strided(
                flat, shape=(9, 9, 552 * B_LOC),
                strides=(28 * B_LOC * st, B_LOC * st, st),
            ).reshape(81, 552 * B_LOC))
        in_maps.append(dict(x81d=x81_np, w1T=w1T_np, b1=b1_np,
                            w2T=w2T_np, b2=b2_np))

    import time as _time
    _t0 = _time.time()
    res = run_bass_kernel_spmd(nc, in_maps, list(range(N_CORES)), trace=_trace)
    _cache["spmd_time"] = _time.time() - _t0
    results = res.results

    # gather u: per core [256, 36*B_LOC] -> u[b, 1152, 8]
    u_parts = []
    for k in range(N_CORES):
        uo = np.asarray(results[k]["u_out"]).reshape(256, 36, B_LOC)
        u_parts.append(uo.transpose(2, 0, 1).reshape(B_LOC, 256 * 36))
    u = np.concatenate(u_parts, 0).reshape(B, 1152, 8).astype(np.float32)

    # ---- routing + decoder on host (tiny FLOPs) ----
    u = _squash(u)
    # u_hat[b,r,c,o] ; batched over r: [r, b, i] @ [r, i, (c o)]
    Wr = W_dc.reshape(1152, 160, 8).transpose(0, 2, 1).astype(np.float32)
    u_hat = np.matmul(u.transpose(1, 0, 2), Wr)  # [1152, b, 160]
    b_ij = np.zeros((1152, 10), np.float32)
    uh2 = u_hat.reshape(1152, B, 10, 16)
    v = None
    for it in range(3):
        c_ij = _softmax(b_ij, axis=0)
        s_j = np.einsum("rc,rbco->bco", c_ij, uh2, optimize=True)
        v = _squash(s_j, axis=-1)
        if it < 2:
            a_ij = np.einsum("rbco,bco->rc", uh2, v, optimize=True) / B
            b_ij = b_ij + a_ij

    output = v[..., None].astype(np.float32)  # [B,10,16,1]
    classes = np.sqrt(np.sum(output ** 2, axis=2))  # [B,10,1]
    classes = _softmax(classes, axis=0)
    idx = np.argmax(classes, axis=1)[:, 0]
    masked = np.eye(10, dtype=np.float32)[idx]
    t = (output * masked[:, :, None, None]).reshape(B, -1)
    h = np.maximum(t @ dec_w1 + dec_b1, 0.0)
    h = np.maximum(h @ dec_w2 + dec_b2, 0.0)
    recon = 1.0 / (1.0 + np.exp(-(h @ dec_w3 + dec_b3)))
    recon = recon.reshape(B, 1, 28, 28).astype(np.float32)

    out = (output, recon, masked)
    if _return_results:
        return out, res
    return out


# revision 13
# speedup vs baseline: 26.3240x; 26.3240x over previous
import sys, os
sys.path.insert(0, "/opt/trn_rl_repo")
import numpy as np

import concourse.bass as bass
import concourse.bacc as bacc
import concourse.mybir as mybir
from concourse.tile import TileContext
from concourse.bass_utils import run_bass_kernel_spmd
from concourse import bass2jax as _b2j
import jax
from jax.sharding import Mesh, PartitionSpec
from jax.experimental.shard_map import shard_map

F32 = mybir.dt.float32
F32R = mybir.dt.float32r

N_CORES = 8
B = 256
B_LOC = B // N_CORES  # 32

# matmul operand dtype mode: "f32r" (full-rate fp32), "f32" (4x slower), "bf16"
MM_MODE = os.environ.get("CAPS_MM_MODE", "f32r")

_cache = {}


def _mm_cast(ap):
    return ap


def build_kernel():
    nc = bacc.Bacc("TRN2", target_bir_lowering=False, debug=False, num_devices=N_CORES)

    x81d = nc.declare_dram_parameter("x81d", [81, 552 * B_LOC], F32, isOutput=False)
    w1T = nc.declare_dram_parameter("w1T", [81, 256], F32, isOutput=False)
    b1 = nc.declare_dram_parameter("b1", [128, 2], F32, isOutput=False)
    w2T = nc.declare_dram_parameter("w2T", [81, 256, 256], F32, isOutput=False)
    b2 = nc.declare_dram_parameter("b2", [128, 2], F32, isOutput=False)
    u_out = nc.declare_dram_parameter("u_out", [256, 36 * B_LOC], F32, isOutput=True)

    with TileContext(nc) as tc:
        with (
            tc.tile_pool(name="const", bufs=1) as cpool,
            tc.tile_pool(name="x81p", bufs=1) as x81pool,
            tc.tile_pool(name="xsb", bufs=1) as xpool,
            tc.tile_pool(name="usb", bufs=1) as upool,
            tc.tile_pool(name="w2p", bufs=4) as w2pool,
        ):
            # ---- constants ----
            w1_sb = cpool.tile([81, 256], F32R)
            nc.gpsimd.dma_start(out=w1_sb[:, :], in_=w1T[:, :])
            b1_sb = cpool.tile([128, 2], F32)
            nc.sync.dma_start(out=b1_sb[:, :], in_=b1[:, :])
            b2_sb = cpool.tile([128, 2], F32)
            nc.sync.dma_start(out=b2_sb[:, :], in_=b2[:, :])

            # ---- conv1 im2col: X81[p=(ky,kx), q'=(oy*28+ox), b] ----
            # partition p reads imgT.flat[(ky*28+kx)*32 : +552*32]
            x81 = x81pool.tile([81, 552 * B_LOC], F32R)
            nc.gpsimd.dma_start(out=x81[:, :], in_=x81d[:, :])

            # ---- conv1: out[co, oy, ox, b], K=81 ----
            x_sb = xpool.tile([128, 2 * 20 * 20 * B_LOC], F32R)  # free=(cot,y,x,b)
            x_r = x_sb.rearrange(
                "p (t y x b) -> p t y x b", t=2, y=20, x=20, b=B_LOC
            )
            x81_r = x81.rearrange("p (q b) -> p q b", b=B_LOC)
            with tc.tile_pool(name="ps1", bufs=4, space="PSUM") as ps1:
                for oy in range(20):
                    for h in range(2):
                        q0 = oy * 28 + h * 10
                        rhs = x81_r[:, q0 : q0 + 10, :]
                        for cot in range(2):
                            pt = ps1.tile([128, 10 * B_LOC], F32)
                            nc.tensor.matmul(
                                pt[:, :],
                                _mm_cast(w1_sb[:, cot * 128 : cot * 128 + 128]),
                                _mm_cast(rhs),
                                start=True,
                                stop=True,
                            )
                            # bias + relu -> SBUF
                            nc.scalar.activation(
                                x_r[:, cot, oy, h * 10 : h * 10 + 10, :],
                                pt[:, :],
                                mybir.ActivationFunctionType.Relu,
                                bias=b1_sb[:, cot : cot + 1],
                            )

            # ---- conv2: 9x9 s2, K=256 (2 tiles), out u[co, s=(6x6), b] ----
            u_sb = upool.tile([128, 2 * 36 * B_LOC], F32)  # free=(cot, s, b)
            u_r = u_sb.rearrange("p (t s b) -> p t s b", t=2, s=36, b=B_LOC)
            # strided view of x for stride-2 conv taps
            x_v = x_sb.rearrange(
                "p (t y2 yp x2 xp b) -> p t yp xp y2 x2 b",
                t=2, y2=10, yp=2, x2=10, xp=2, b=B_LOC,
            )
            with tc.tile_pool(name="ps2", bufs=1, space="PSUM") as ps2:
                pts = {}
                for cot in range(2):
                    for j in range(3):
                        pts[(cot, j)] = ps2.tile([128, 6 * 2 * B_LOC], F32, tag=f"acc{cot}{j}", name=f"acc{cot}{j}")
                for pos in range(81):
                    ky, kx = pos // 9, pos % 9
                    wt = w2pool.tile([128, 2 * 256], F32R, tag="w2")
                    wsrc = w2T.rearrange("pos (ct p) co -> pos p ct co", ct=2, p=128)
                    nc.gpsimd.dma_start(out=wt[:, :], in_=wsrc[pos, :, :, :])
                    wt_r = wt.rearrange("p (ct co) -> p ct co", ct=2)
                    for cint in range(2):
                        for cot in range(2):
                            lhsT = wt_r[:, cint, cot * 128 : cot * 128 + 128]
                            for j in range(3):
                                rhs = x_v[
                                    :, cint, ky % 2, kx % 2,
                                    ky // 2 : ky // 2 + 6,
                                    kx // 2 + 2 * j : kx // 2 + 2 * j + 2,
                                    :,
                                ]
                                nc.tensor.matmul(
                                    pts[(cot, j)][:, :],
                                    _mm_cast(lhsT),
                                    _mm_cast(rhs),
                                    start=(pos == 0 and cint == 0),
                                    stop=(pos == 80 and cint == 1),
                                )
                # epilogue: bias add, to u_sb; s=(oy*6+ox), chunk j covers ox'(2j,2j+1)
                u_v = u_sb.rearrange(
                    "p (t oy j ox b) -> p t oy j ox b", t=2, oy=6, j=3, ox=2, b=B_LOC
                )
                for cot in range(2):
                    for j in range(3):
                        src_p = pts[(cot, j)].rearrange(
                            "p (oy ox b) -> p oy ox b", oy=6, ox=2, b=B_LOC
                        )
                        nc.vector.tensor_scalar_add(
                            u_v[:, cot, :, j, :, :],
                            src_p[:, :, :, :],
                            b2_sb[:, cot : cot + 1],
                        )

            # ---- store u ----
            uo = u_out.rearrange("(t p) sb -> p t sb", t=2, p=128)
            nc.sync.dma_start(out=uo[:, :, :], in_=u_sb[:, :])

    nc.finalize()
    return nc


def _get_nc():
    if "nc" not in _cache:
        _cache["nc"] = build_kernel()
    return _cache["nc"]


def _get_runner():
    # cached jitted shard_map over the bass_exec custom call: avoids
    # re-lowering through run_bass_via_pjrt on every kernel() invocation
    if "runner" in _cache:
        return _cache["runner"]
    nc = _get_nc()
    _b2j.install_neuronx_cc_hook()
    import concourse.mybir as _mybir
    partition_name = nc.partition_id_tensor.name if nc.partition_id_tensor else None
    in_names, out_names, out_avals, zero_outs = [], [], [], []
    for alloc in nc.m.functions[0].allocations:
        if not isinstance(alloc, _mybir.MemoryLocationSet):
            continue
        name = alloc.memorylocations[0].name
        if alloc.kind == "ExternalInput":
            if name != partition_name:
                in_names.append(name)
        elif alloc.kind == "ExternalOutput":
            shape = tuple(alloc.tensor_shape)
            dtype = _mybir.dt.np(alloc.dtype)
            out_names.append(name)
            out_avals.append(jax.core.ShapedArray(shape, dtype))
            zero_outs.append(np.zeros(shape, dtype))
    n_params = len(in_names)
    n_outs = len(out_avals)
    in_names_all = in_names + out_names
    if partition_name is not None:
        in_names_all.append(partition_name)
    donate = tuple(range(n_params, n_params + n_outs))

    def _body(*args):
        operands = list(args)
        if partition_name is not None:
            operands.append(_b2j.partition_id_tensor())
        outs = _b2j._bass_exec_p.bind(
            *operands,
            out_avals=tuple(out_avals),
            in_names=tuple(in_names_all),
            out_names=tuple(out_names),
            lowering_input_output_aliases=(),
            sim_require_finite=True,
            sim_require_nnan=True,
            nc=nc,
        )
        return tuple(outs)

    devices = jax.devices()[:N_CORES]
    mesh = Mesh(np.asarray(devices), ("core",))
    in_specs = (PartitionSpec("core"),) * (n_params + n_outs)
    out_specs = (PartitionSpec("core"),) * n_outs
    sharded = jax.jit(
        shard_map(_body, mesh=mesh, in_specs=in_specs, out_specs=out_specs,
                  check_rep=False),
        donate_argnums=donate, keep_unused=True,
    )
    _cache["runner"] = (sharded, in_names, out_names, out_avals, zero_outs)
    return _cache["runner"]


def run_spmd(in_maps):
    sharded, in_names, out_names, out_avals, zero_outs = _get_runner()
    n = N_CORES
    concat_in = [
        np.concatenate([np.asarray(in_maps[c][name]) for c in range(n)], axis=0)
        for name in in_names
    ]
    concat_zeros = [
        np.zeros((n * z.shape[0], *z.shape[1:]), z.dtype) for z in zero_outs
    ]
    out_arrs = sharded(*concat_in, *concat_zeros)
    return [
        {name: np.asarray(out_arrs[i]).reshape(n, *out_avals[i].shape)[c]
         for i, name in enumerate(out_names)}
        for c in range(n)
    ]


def _softmax(x, axis):
    m = np.max(x, axis=axis, keepdims=True)
    e = np.exp(x - m)
    return e / np.sum(e, axis=axis, keepdims=True)


def _squash(x, axis=-1):
    sq = np.sum(x * x, axis=axis, keepdims=True)
    return sq * x / ((1.0 + sq) * np.sqrt(sq))


def kernel(data, conv1_w, conv1_b, pc_w, pc_b, W_dc,
           dec_w1, dec_b1, dec_w2, dec_b2, dec_w3, dec_b3,
           _return_results=False, _trace=False):
    nc = _get_nc()

    w1T_np = np.ascontiguousarray(
        conv1_w.reshape(256, 81).T.astype(np.float32))
    b1_np = np.ascontiguousarray(conv1_b.reshape(2, 128).T.astype(np.float32))
    w2T_np = np.ascontiguousarray(
        pc_w.reshape(256, 256, 9, 9).transpose(2, 3, 1, 0).reshape(81, 256, 256)
        .astype(np.float32))
    b2_np = np.ascontiguousarray(pc_b.reshape(2, 128).T.astype(np.float32))

    in_maps = []
    for k in range(N_CORES):
        sl = data[k * B_LOC : (k + 1) * B_LOC].reshape(B_LOC, 784)
        imgT_np = np.ascontiguousarray(sl.T.astype(np.float32))
        flat = imgT_np.reshape(-1)
        st = flat.strides[0]
        x81_np = np.ascontiguousarray(
            np.lib.stride_tricks.as_strided(
                flat, shape=(9, 9, 552 * B_LOC),
                strides=(28 * B_LOC * st, B_LOC * st, st),
            ).reshape(81, 552 * B_LOC))
        in_maps.append(dict(x81d=x81_np, w1T=w1T_np, b1=b1_np,
                            w2T=w2T_np, b2=b2_np))
    _cache["last_in_maps"] = in_maps

    import time as _time
    _t0 = _time.time()
    results = run_spmd(in_maps)
    _cache["spmd_time"] = _time.time() - _t0
    res = None

    # gather u: per core [256, 36*B_LOC] -> u[b, 1152, 8]
    u_parts = []
    for k in range(N_CORES):
        uo = np.asarray(results[k]["u_out"]).reshape(256, 36, B_LOC)
        u_parts.append(uo.transpose(2, 0, 1).reshape(B_LOC, 256 * 36))
    u = np.concatenate(u_parts, 0).reshape(B, 1152, 8).astype(np.float32)

    # ---- routing + decoder on host (tiny FLOPs) ----
    u = _squash(u)
    # u_hat[b,r,c,o] ; batched over r: [r, b, i] @ [r, i, (c o)]
    Wr = W_dc.reshape(1152, 160, 8).transpose(0, 2, 1).astype(np.float32)
    u_hat = np.matmul(u.transpose(1, 0, 2), Wr)  # [1152, b, 160]
    b_ij = np.zeros((1152, 10), np.float32)
    uh2 = u_hat.reshape(1152, B, 10, 16)
    v = None
    for it in range(3):
        c_ij = _softmax(b_ij, axis=0)
        s_j = np.einsum("rc,rbco->bco", c_ij, uh2, optimize=True)
        v = _squash(s_j, axis=-1)
        if it < 2:
            a_ij = np.einsum("rbco,bco->rc", uh2, v, optimize=True) / B
            b_ij = b_ij + a_ij

    output = v[..., None].astype(np.float32)  # [B,10,16,1]
    classes = np.sqrt(np.sum(output ** 2, axis=2))  # [B,10,1]
    classes = _softmax(classes, axis=0)
    idx = np.argmax(classes, axis=1)[:, 0]
    masked = np.eye(10, dtype=np.float32)[idx]
    t = (output * masked[:, :, None, None]).reshape(B, -1)
    h = np.maximum(t @ dec_w1 + dec_b1, 0.0)
    h = np.maximum(h @ dec_w2 + dec_b2, 0.0)
    recon = 1.0 / (1.0 + np.exp(-(h @ dec_w3 + dec_b3)))
    recon = recon.reshape(B, 1, 28, 28).astype(np.float32)

    out = (output, recon, masked)
    if _return_results:
        return out, None
    return out
